# revision 1
# baseline (speedup 1.0000x reference)
"""Distributed segment-max (BatchPooling) for 8 Trainium2 NeuronCores.

Strategy (data/segment parallel, per the sharding hint):
  - Split the node dim N into 8 contiguous row shards, one per core.
  - On each core, compute the max over every aligned K=128-row block of its
    shard.  Chunks of 16384 consecutive rows (8 MiB) are DMA'd so that
    partition p holds rows [chunk*16384 + p*128, +128) — all 128 partition
    streams sit inside one contiguous 8 MiB region, which measures ~354 GB/s
    per core with all 8 cores running (HBM-domain saturation: 2 cores share
    ~716 GB/s) vs ~293 GB/s for a partition-strided layout.
  - The per-block max is a single free-axis `reduce_max` per chunk with a
    strided access pattern (feature dim stride 1, row dim stride D) — no
    transposes anywhere.  DVE work (~137 us/core) hides under the DMA.
  - The host folds block maxes into segment maxes.  For the uniform layout
    produced by the reference (segments of 512 = 4 blocks) this is an exact
    reshape+max; for general sorted `batch` the few rows at non-aligned
    segment edges are fixed up from x directly.  max is associative /
    idempotent and involves no arithmetic, so the result is bit-exact.

Raw bass (not Tile) because a recycling load DMA needs two waits (WAR on
DVE + WAW on the previous load) and the PSEUDO_DMA_DIRECT2D lowering only
supports one inline wait; standalone sequencer `wait_ge` instructions
sidestep that.  The WAW on a recycled buffer is implied transitively:
red_sem >= readers-of-that-buffer means those reduces ran, and they only
ran after observing the previous load's dma_sem increment.

The per-core kernel reads 64 MiB at the measured HBM rate — the memory
roofline for this problem (target_regime=memory).
"""

import contextlib

import numpy as np

_P = 128  # SBUF partitions
_D = 128  # feature dim (hardcoded per problem spec)
_K = 128  # rows per device-reduced block (one block per chunk per partition)
_NCORES = 8
_CHUNK_ROWS = 128  # rows per partition per DMA chunk

_CACHE = {}


def _build_nc(
    rows_per_core,
    repeats=1,
    bufs=2,
    chunk_rows=None,
    split_store=True,
    block_rows=None,
    tail_split=4,
    gp_fold=False,
):
    """One NeuronCore's program: SP streams contiguous chunks, DVE reduces
    each partition's `chunk_rows` rows to one block max, block maxes are
    stored (either once at the end on SP, or per-chunk on the ACT ring when
    `split_store` so the store latency hides under the loads).

    `repeats` re-runs the whole pipeline (used by the timing harness to
    isolate HW time via wall-clock deltas); the double-buffer rotation spans
    repeats so the steady state matches a larger input.
    """
    import concourse.bass as bass
    import concourse.mybir as mybir

    if chunk_rows is None:
        chunk_rows = _CHUNK_ROWS
    if block_rows is None:
        block_rows = _K
    nc = bass.Bass()
    rows_per_part = rows_per_core // _P
    n_chunks = rows_per_part // chunk_rows
    n_blocks = rows_per_core // block_rows
    bpc = chunk_rows // block_rows  # blocks per chunk per partition

    x = nc.dram_tensor("x", [rows_per_core, _D], mybir.dt.float32, kind="ExternalInput")
    bm = nc.dram_tensor("bm", [n_blocks, _D], mybir.dt.float32, kind="ExternalOutput")

    # Chunk c = contiguous rows [c*P*CR, (c+1)*P*CR); partition p takes the
    # p-th CR-row run inside it, i.e. bpc consecutive K-row blocks.
    xc = x[:].rearrange("(c p w) d -> c p (w d)", c=n_chunks, p=_P)
    # Sequential block index = c*(P*bpc) + p*bpc + b.
    bo = bm[:].rearrange("(c p b) d -> c p (b d)", c=n_chunks, p=_P)

    cw = chunk_rows * _D  # elements per partition per chunk

    with contextlib.ExitStack() as es:
        tiles = es.enter_context(nc.sbuf_tensor([_P, bufs * cw], mybir.dt.float32))
        bmt = es.enter_context(
            nc.sbuf_tensor([_P, n_chunks * bpc * _D], mybir.dt.float32)
        )
        # One DMA-completion sem per buffer slot: at most one in-flight DMA
        # per sem, so `sem >= 16*(k+1)` exactly means "the k-th load into
        # this slot fully landed" (a single cumulative sem could pass its
        # threshold early if SDMA engines progress unevenly across chunks).
        dma_sems = [
            es.enter_context(nc.semaphore(f"dma_sem{i}")) for i in range(bufs)
        ]
        store_sem = es.enter_context(nc.semaphore("store_sem"))
        red_sem = es.enter_context(nc.semaphore("red_sem"))
        block = es.enter_context(nc.Block())

        cbw = bpc * _D  # bmt elements per chunk

        # The last chunk of each sweep is optionally loaded as `tail_split`
        # sub-pieces so DVE can reduce partials while the remaining bytes
        # stream in — the post-last-byte serial tail shrinks from a full
        # chunk reduce to one sub-reduce plus a short tensor_max chain.
        use_tail = tail_split > 1 and bpc == 1 and chunk_rows % tail_split == 0
        sub_rows = chunk_rows // tail_split if use_tail else 0
        sub_w = sub_rows * _D
        if use_tail:
            sub_sems = [
                es.enter_context(nc.semaphore(f"sub_sem{i}"))
                for i in range(tail_split)
            ]
            tmps = es.enter_context(
                nc.sbuf_tensor([_P, tail_split * _D], mybir.dt.float32)
            )
        # per-parity ordinal of each full-chunk load (tail loads use their
        # own sems, so dma_sems counting must skip tail chunks)
        full_ordinal = {}
        gp_ordinal = {}
        counts = [0] * bufs
        for g in range(repeats * n_chunks):
            c = g % n_chunks
            if use_tail and c == n_chunks - 1:
                continue
            counts[g % bufs] += 1
            full_ordinal[g] = counts[g % bufs]
            gp_ordinal[g] = len(gp_ordinal) + 1

        # gp_fold: GPSIMD pre-folds each full chunk's two contiguous halves
        # (rows p*CR+i with p*CR+CR/2+i — same K-row block) so DVE only
        # reduces half the elements; DMA becomes the pacing engine.
        use_gp = gp_fold and bpc == 1 and chunk_rows % 2 == 0
        if use_gp:
            gp_sem = es.enter_context(nc.semaphore("gp_sem"))

            @block.gpsimd
            def _(gpsimd):
                for r in range(repeats):
                    for c in range(n_chunks - 1 if use_tail else n_chunks):
                        g = r * n_chunks + c
                        gpsimd.wait_ge(dma_sems[g % bufs], 16 * full_ordinal[g])
                        slot = (g % bufs) * cw
                        half = cw // 2
                        nc.gpsimd.tensor_max(
                            out=tiles[:, slot : slot + half],
                            in0=tiles[:, slot : slot + half],
                            in1=tiles[:, slot + half : slot + cw],
                        ).then_inc(gp_sem, 1)

        @block.sync
        def _(sync):
            for r in range(repeats):
                for c in range(n_chunks):
                    g = r * n_chunks + c
                    if g >= bufs:
                        # the previous tenant of this slot has been reduced
                        # (which also implies that load fully landed)
                        sync.wait_ge(red_sem, bpc * (g - bufs + 1))
                    slot = (g % bufs) * cw
                    if use_tail and c == n_chunks - 1:
                        # sub-piece i = rows [p*CR + i*sub_rows, +sub_rows) of
                        # each partition's run — a column slice of the chunk
                        # view, so the block row-sets are unchanged
                        for i in range(tail_split):
                            sync.dma_start(
                                out=tiles[:, slot + i * sub_w : slot + (i + 1) * sub_w],
                                in_=xc[c][:, i * sub_w : (i + 1) * sub_w],
                            ).then_inc(sub_sems[i], 16)
                    else:
                        sync.dma_start(
                            out=tiles[:, slot : slot + cw], in_=xc[c]
                        ).then_inc(dma_sems[g % bufs], 16)
                if not split_store:
                    sync.wait_ge(red_sem, bpc * n_chunks * (r + 1))
                    sync.dma_start(
                        out=bm[:].rearrange(
                            "(c p b) d -> p c (b d)", c=n_chunks, p=_P
                        ),
                        in_=bmt[:].rearrange("p (c w) -> p c w", c=n_chunks),
                    ).then_inc(store_sem, 16)
            if not split_store:
                sync.wait_ge(store_sem, 16 * repeats)

        if split_store:
            # per-chunk block-max stores ride the ACT HWDGE ring so they
            # never queue behind loads on the SP ring
            @block.scalar
            def _(scalar):
                for r in range(repeats):
                    for c in range(n_chunks):
                        g = r * n_chunks + c
                        scalar.wait_ge(red_sem, bpc * (g + 1))
                        scalar.dma_start(
                            out=bo[c], in_=bmt[:, c * cbw : (c + 1) * cbw]
                        ).then_inc(store_sem, 16)
                scalar.wait_ge(store_sem, 16 * repeats * n_chunks)

        @block.vector
        def _(vector):
            for r in range(repeats):
                for c in range(n_chunks):
                    g = r * n_chunks + c
                    if r > 0:
                        # bmt WAR: the store that read this bmt slice in
                        # repeat r-1 must be done before we overwrite it
                        if split_store:
                            vector.wait_ge(store_sem, 16 * ((r - 1) * n_chunks + c + 1))
                        elif c == 0:
                            vector.wait_ge(store_sem, 16 * r)
                    slot = (g % bufs) * cw
                    if use_tail and c == n_chunks - 1:
                        for i in range(tail_split):
                            vector.wait_ge(sub_sems[i], 16 * (r + 1))
                            sv = tiles[
                                :, slot + i * sub_w : slot + (i + 1) * sub_w
                            ].rearrange("p (m d) -> p d m", m=sub_rows, d=_D)
                            nc.vector.reduce_max(
                                out=tmps[:, i * _D : (i + 1) * _D],
                                in_=sv,
                                axis=mybir.AxisListType.X,
                            )
                        # pairwise fold the partial maxes into the block max
                        live = list(range(tail_split))
                        while len(live) > 2:
                            nxt = []
                            for j in range(0, len(live) - 1, 2):
                                a, b2 = live[j], live[j + 1]
                                nc.vector.tensor_max(
                                    out=tmps[:, a * _D : (a + 1) * _D],
                                    in0=tmps[:, a * _D : (a + 1) * _D],
                                    in1=tmps[:, b2 * _D : (b2 + 1) * _D],
                                )
                                nxt.append(a)
                            if len(live) % 2:
                                nxt.append(live[-1])
                            live = nxt
                        nc.vector.tensor_max(
                            out=bmt[:, c * _D : (c + 1) * _D],
                            in0=tmps[:, live[0] * _D : (live[0] + 1) * _D],
                            in1=tmps[:, live[1] * _D : (live[1] + 1) * _D],
                        ).then_inc(red_sem, 1)
                    elif use_gp:
                        vector.wait_ge(gp_sem, gp_ordinal[g])
                        view = tiles[:, slot : slot + cw // 2].rearrange(
                            "p (m d) -> p d m", m=chunk_rows // 2, d=_D
                        )
                        nc.vector.reduce_max(
                            out=bmt[:, c * _D : (c + 1) * _D],
                            in_=view,
                            axis=mybir.AxisListType.X,
                        ).then_inc(red_sem, 1)
                    else:
                        vector.wait_ge(
                            dma_sems[g % bufs], 16 * full_ordinal[g]
                        )
                        view = tiles[:, slot : slot + cw].rearrange(
                            "p (b m d) -> p b d m", b=bpc, m=block_rows, d=_D
                        )
                        for b in range(bpc):
                            nc.vector.reduce_max(
                                out=bmt[
                                    :, (c * bpc + b) * _D : (c * bpc + b + 1) * _D
                                ],
                                in_=view[:, b],
                                axis=mybir.AxisListType.X,
                            ).then_inc(red_sem, 1)
    return nc


def _device_block_max(x):
    from concourse.bass_utils import run_bass_kernel_spmd

    n = x.shape[0]
    rows_per_core = n // _NCORES
    if rows_per_core not in _CACHE:
        _CACHE[rows_per_core] = _build_nc(rows_per_core)
    nc = _CACHE[rows_per_core]
    shards = [x[i * rows_per_core : (i + 1) * rows_per_core] for i in range(_NCORES)]
    res = run_bass_kernel_spmd(
        nc, [{"x": s} for s in shards], core_ids=list(range(_NCORES))
    )
    return np.concatenate([r["bm"] for r in res.results], axis=0)


def _combine(bm, x, batch, num_segments):
    n, d = x.shape
    counts = np.bincount(batch, minlength=num_segments)
    starts = np.empty(num_segments + 1, np.int64)
    starts[0] = 0
    np.cumsum(counts, out=starts[1:])

    rows_per_seg = n // num_segments if num_segments else 0
    if (
        num_segments
        and n % num_segments == 0
        and rows_per_seg % _K == 0
        and np.all(counts == rows_per_seg)
    ):
        return np.ascontiguousarray(
            bm.reshape(num_segments, rows_per_seg // _K, d).max(axis=1)
        )

    out = np.full((num_segments, d), -np.inf, dtype=np.float32)
    for s in range(num_segments):
        a, b = int(starts[s]), int(starts[s + 1])
        if a >= b:
            continue
        ca, cb = -(-a // _K), b // _K
        best = None
        if ca < cb:
            best = bm[ca:cb].max(axis=0)
        lo_end = min(b, ca * _K)
        if a < lo_end:
            e = x[a:lo_end].max(axis=0)
            best = e if best is None else np.maximum(best, e)
        hi_start = max(a, cb * _K)
        if hi_start < b:
            e = x[hi_start:b].max(axis=0)
            best = e if best is None else np.maximum(best, e)
        out[s] = best
    return out


def _numpy_segment_max(x, batch, num_segments):
    """Pure-host fallback for inputs the device path doesn't cover
    (unsorted batch, out-of-range ids, unexpected shapes)."""
    out = np.full((num_segments, x.shape[1]), -np.inf, dtype=np.float32)
    if batch.size == 0 or num_segments == 0:
        return out
    keep = (batch >= 0) & (batch < num_segments)
    xb, bb = x[keep], batch[keep]
    order = np.argsort(bb, kind="stable")
    xb, bb = xb[order], bb[order]
    counts = np.bincount(bb, minlength=num_segments)
    starts = np.concatenate([[0], np.cumsum(counts)[:-1]])
    nonempty = counts > 0
    idx = starts[nonempty]
    if idx.size:
        out[nonempty] = np.maximum.reduceat(xb, idx, axis=0)
    return out


def kernel(x, batch, num_segments):
    x = np.ascontiguousarray(np.asarray(x), dtype=np.float32)
    batch = np.asarray(batch)
    num_segments = int(np.asarray(num_segments))
    n, d = x.shape

    in_range = batch.size == 0 or (
        int(batch[0]) >= 0 and int(batch[-1]) < num_segments
    )
    sorted_ok = batch.size == 0 or bool(np.all(batch[1:] >= batch[:-1]))
    shape_ok = d == _D and n == batch.shape[0] and n % (_NCORES * _P * _CHUNK_ROWS) == 0

    if not (shape_ok and sorted_ok and in_range):
        return _numpy_segment_max(x, batch, num_segments)

    bm = _device_block_max(x)
    return _combine(bm, x, batch, num_segments)



# revision 3
# speedup vs baseline: 1.3795x; 1.3795x over previous
"""Distributed segment-max (BatchPooling) for 8 Trainium2 NeuronCores.

Strategy (data/segment parallel, per the sharding hint):
  - Split the node dim N into 8 contiguous row shards, one per core.
  - On each core, compute the max over every aligned K=128-row block of its
    shard.  Chunks of 16384 consecutive rows (8 MiB) are DMA'd so that
    partition p holds rows [chunk*16384 + p*128, +128) — all 128 partition
    streams sit inside one contiguous 8 MiB region.  A single SP-ring load
    stream measures ~341 GB/s/core with all 8 cores running; adding the ACT
    ring or SWDGE does not raise it (hard per-core ceiling), so loads stay
    on one ring.
  - Each partition's 128-row block is reduced with a unit-stride in-place
    tensor_max fold tree (16384 -> 128 elements in 7 elementwise maxes,
    every operand contiguous).  Measured DVE busy: ~141 us/sweep/core vs
    ~220 us/sweep for the strided reduce_max view (the DVE runs the d-major
    strided pattern at ~0.43 elem/cycle/lane, which made the old reduce the
    pipeline bottleneck).  At 141 us the DVE hides under the ~197 us DMA
    stream; the full pipeline measures ~200 us/sweep/core vs the 197 us
    loads-only ceiling.
  - Block maxes accumulate into a sweep-parity half of a small bmt tile and
    are stored once per sweep on the ACT HWDGE ring (never queued behind
    loads).  The parity double-buffer means the DVE almost never waits on
    store completion.
  - The host folds block maxes into segment maxes.  For the uniform layout
    produced by the reference (segments of 512 = 4 blocks) this is an exact
    reshape+max; for general sorted `batch` the few rows at non-aligned
    segment edges are fixed up from x directly.  max is associative /
    idempotent and involves no arithmetic, so the result is bit-exact.

Raw bass (not Tile): the pipeline needs two waits in front of some DMAs
(WAR on DVE progress + WAW on the previous tenant) and explicit sem
threshold arithmetic; standalone sequencer wait_ge instructions express
this directly.

`repeats` re-runs the whole pipeline (used by the timing harness to
isolate HW time via wall-clock deltas); buffer rotation spans repeats so
the steady state matches a larger input.
"""

import contextlib

import numpy as np

_P = 128  # SBUF partitions
_D = 128  # feature dim (hardcoded per problem spec)
_K = 128  # rows per device-reduced block (one block per chunk per partition)
_NCORES = 8
_CHUNK_ROWS = 128  # rows per partition per DMA chunk

_CACHE = {}


def _build_nc(rows_per_core, repeats=1, bufs=3, chunk_rows=None):
    """One NeuronCore's program: SP streams contiguous 8 MiB chunks, DVE
    folds each partition's rows to a block max via a unit-stride in-place
    tensor_max tree, ACT stores each sweep's block maxes."""
    import concourse.bass as bass
    import concourse.mybir as mybir

    if chunk_rows is None:
        chunk_rows = _CHUNK_ROWS
    nc = bass.Bass()
    rows_per_part = rows_per_core // _P
    n_chunks = rows_per_part // chunk_rows
    n_blocks = rows_per_core // chunk_rows
    cw = chunk_rows * _D  # elements per partition per chunk

    x = nc.dram_tensor("x", [rows_per_core, _D], mybir.dt.float32, kind="ExternalInput")
    bm = nc.dram_tensor("bm", [n_blocks, _D], mybir.dt.float32, kind="ExternalOutput")

    # Chunk c = contiguous rows [c*P*CR, (c+1)*P*CR); partition p takes the
    # p-th CR-row run inside it.  Sequential block index = c*P + p.
    xc = x[:].rearrange("(c p w) d -> c p (w d)", c=n_chunks, p=_P)
    # Whole-sweep store view: partition p, then chunk-major, d inner.
    bo = bm[:].rearrange("(c p) d -> p c d", c=n_chunks, p=_P)

    total = repeats * n_chunks
    sw = n_chunks * _D  # bmt elements per sweep half

    with contextlib.ExitStack() as es:
        tiles = es.enter_context(nc.sbuf_tensor([_P, bufs * cw], mybir.dt.float32))
        bmt = es.enter_context(nc.sbuf_tensor([_P, 2 * sw], mybir.dt.float32))
        # One DMA-completion sem per buffer slot: at most one in-flight DMA
        # per sem, so `sem >= 16*k` exactly means "the k-th load into this
        # slot fully landed" (a single cumulative sem could pass its
        # threshold early if SDMA engines progress unevenly across chunks).
        dma_sems = [
            es.enter_context(nc.semaphore(f"dma_sem{i}")) for i in range(bufs)
        ]
        store_sem = es.enter_context(nc.semaphore("store_sem"))
        red_sem = es.enter_context(nc.semaphore("red_sem"))
        block = es.enter_context(nc.Block())

        # per-slot ordinal of each load (for the slot's completion sem)
        ordinal = [0] * bufs
        ords = {}
        for g in range(total):
            ordinal[g % bufs] += 1
            ords[g] = ordinal[g % bufs]

        @block.sync
        def _(sync):
            for g in range(total):
                if g >= bufs:
                    # the previous tenant of this slot has been folded
                    # (which also implies that load fully landed)
                    sync.wait_ge(red_sem, g - bufs + 1)
                slot = (g % bufs) * cw
                sync.dma_start(
                    out=tiles[:, slot : slot + cw], in_=xc[g % n_chunks]
                ).then_inc(dma_sems[g % bufs], 16)
            sync.wait_ge(store_sem, 16 * repeats)

        @block.scalar
        def _(scalar):
            for r in range(repeats):
                scalar.wait_ge(red_sem, n_chunks * (r + 1))
                half = (r % 2) * sw
                scalar.dma_start(
                    out=bo, in_=bmt[:, half : half + sw]
                ).then_inc(store_sem, 16)

        @block.vector
        def _(vector):
            for g in range(total):
                r, c = divmod(g, n_chunks)
                if r >= 2 and c == 0:
                    # bmt WAR: this parity half was last read by the store
                    # of sweep r-2
                    vector.wait_ge(store_sem, 16 * (r - 1))
                vector.wait_ge(dma_sems[g % bufs], 16 * ords[g])
                slot = (g % bufs) * cw
                half = (r % 2) * sw
                # in-place unit-stride fold tree: rows [0,m/2) vs [m/2,m)
                m = chunk_rows
                while m > 2:
                    h = (m // 2) * _D
                    nc.vector.tensor_max(
                        out=tiles[:, slot : slot + h],
                        in0=tiles[:, slot : slot + h],
                        in1=tiles[:, slot + h : slot + 2 * h],
                    )
                    m //= 2
                nc.vector.tensor_max(
                    out=bmt[:, half + c * _D : half + (c + 1) * _D],
                    in0=tiles[:, slot : slot + _D],
                    in1=tiles[:, slot + _D : slot + 2 * _D],
                ).then_inc(red_sem, 1)
    return nc


def _device_block_max(x):
    from concourse.bass_utils import run_bass_kernel_spmd

    n = x.shape[0]
    rows_per_core = n // _NCORES
    if rows_per_core not in _CACHE:
        _CACHE[rows_per_core] = _build_nc(rows_per_core)
    nc = _CACHE[rows_per_core]
    shards = [x[i * rows_per_core : (i + 1) * rows_per_core] for i in range(_NCORES)]
    res = run_bass_kernel_spmd(
        nc, [{"x": s} for s in shards], core_ids=list(range(_NCORES))
    )
    return np.concatenate([r["bm"] for r in res.results], axis=0)


def _combine(bm, x, batch, num_segments):
    n, d = x.shape
    counts = np.bincount(batch, minlength=num_segments)
    starts = np.empty(num_segments + 1, np.int64)
    starts[0] = 0
    np.cumsum(counts, out=starts[1:])

    rows_per_seg = n // num_segments if num_segments else 0
    if (
        num_segments
        and n % num_segments == 0
        and rows_per_seg % _K == 0
        and np.all(counts == rows_per_seg)
    ):
        return np.ascontiguousarray(
            bm.reshape(num_segments, rows_per_seg // _K, d).max(axis=1)
        )

    out = np.full((num_segments, d), -np.inf, dtype=np.float32)
    for s in range(num_segments):
        a, b = int(starts[s]), int(starts[s + 1])
        if a >= b:
            continue
        ca, cb = -(-a // _K), b // _K
        best = None
        if ca < cb:
            best = bm[ca:cb].max(axis=0)
        lo_end = min(b, ca * _K)
        if a < lo_end:
            e = x[a:lo_end].max(axis=0)
            best = e if best is None else np.maximum(best, e)
        hi_start = max(a, cb * _K)
        if hi_start < b:
            e = x[hi_start:b].max(axis=0)
            best = e if best is None else np.maximum(best, e)
        out[s] = best
    return out


def _numpy_segment_max(x, batch, num_segments):
    """Pure-host fallback for inputs the device path doesn't cover
    (unsorted batch, out-of-range ids, unexpected shapes)."""
    out = np.full((num_segments, x.shape[1]), -np.inf, dtype=np.float32)
    if batch.size == 0 or num_segments == 0:
        return out
    keep = (batch >= 0) & (batch < num_segments)
    xb, bb = x[keep], batch[keep]
    order = np.argsort(bb, kind="stable")
    xb, bb = xb[order], bb[order]
    counts = np.bincount(bb, minlength=num_segments)
    starts = np.concatenate([[0], np.cumsum(counts)[:-1]])
    nonempty = counts > 0
    idx = starts[nonempty]
    if idx.size:
        out[nonempty] = np.maximum.reduceat(xb, idx, axis=0)
    return out


def kernel(x, batch, num_segments):
    x = np.ascontiguousarray(np.asarray(x), dtype=np.float32)
    batch = np.asarray(batch)
    num_segments = int(np.asarray(num_segments))
    n, d = x.shape

    in_range = batch.size == 0 or (
        int(batch[0]) >= 0 and int(batch[-1]) < num_segments
    )
    sorted_ok = batch.size == 0 or bool(np.all(batch[1:] >= batch[:-1]))
    shape_ok = d == _D and n == batch.shape[0] and n % (_NCORES * _P * _CHUNK_ROWS) == 0

    if not (shape_ok and sorted_ok and in_range):
        return _numpy_segment_max(x, batch, num_segments)

    bm = _device_block_max(x)
    return _combine(bm, x, batch, num_segments)


# revision 4
# speedup vs baseline: 1.4062x; 1.0194x over previous
"""Distributed segment-max (BatchPooling) for 8 Trainium2 NeuronCores.

Strategy (data/segment parallel, per the sharding hint):
  - Split the node dim N into 8 contiguous row shards, one per core.
  - On each core, compute the max over every aligned K=128-row block of its
    shard.  Chunks of 16384 consecutive rows (8 MiB) are DMA'd so that
    partition p holds rows [chunk*16384 + p*128, +128) — all 128 partition
    streams sit inside one contiguous 8 MiB region.  A single SP-ring load
    stream measures ~341 GB/s/core with all 8 cores running; adding the ACT
    ring or SWDGE does not raise it (hard per-core ceiling), so loads stay
    on one ring.
  - Each partition's 128-row block is reduced with a unit-stride in-place
    tensor_max fold tree (16384 -> 128 elements in 7 elementwise maxes,
    every operand contiguous).  Measured DVE busy: ~141 us/sweep/core vs
    ~220 us/sweep for the strided reduce_max view (the DVE runs the d-major
    strided pattern at ~0.43 elem/cycle/lane, which made the old reduce the
    pipeline bottleneck).  At 141 us the DVE hides under the ~192 us DMA
    stream; the full pipeline measures ~195 us/sweep/core vs the ~192 us
    loads-only ceiling (vs 216 us for the strided-reduce baseline).
  - Block maxes accumulate into a sweep-parity half of a small bmt tile and
    are stored once per sweep on the ACT HWDGE ring (never queued behind
    loads).  The parity double-buffer means the DVE almost never waits on
    store completion.
  - The host folds block maxes into segment maxes.  For the uniform layout
    produced by the reference (segments of 512 = 4 blocks) this is an exact
    reshape+max; for general sorted `batch` the few rows at non-aligned
    segment edges are fixed up from x directly.  max is associative /
    idempotent and involves no arithmetic, so the result is bit-exact.

Raw bass (not Tile): the pipeline needs two waits in front of some DMAs
(WAR on DVE progress + WAW on the previous tenant) and explicit sem
threshold arithmetic; standalone sequencer wait_ge instructions express
this directly.

`repeats` re-runs the whole pipeline (used by the timing harness to
isolate HW time via wall-clock deltas); buffer rotation spans repeats so
the steady state matches a larger input.
"""

import contextlib

import numpy as np

_P = 128  # SBUF partitions
_D = 128  # feature dim (hardcoded per problem spec)
_K = 128  # rows per device-reduced block (one block per chunk per partition)
_NCORES = 8
_CHUNK_ROWS = 128  # rows per partition per DMA chunk

_CACHE = {}


def _build_nc(rows_per_core, repeats=1, bufs=3, chunk_rows=None):
    """One NeuronCore's program: SP streams contiguous 8 MiB chunks, DVE
    folds each partition's rows to a block max via a unit-stride in-place
    tensor_max tree, ACT stores each sweep's block maxes."""
    import concourse.bass as bass
    import concourse.mybir as mybir

    if chunk_rows is None:
        chunk_rows = _CHUNK_ROWS
    nc = bass.Bass()
    rows_per_part = rows_per_core // _P
    n_chunks = rows_per_part // chunk_rows
    n_blocks = rows_per_core // chunk_rows
    cw = chunk_rows * _D  # elements per partition per chunk

    x = nc.dram_tensor("x", [rows_per_core, _D], mybir.dt.float32, kind="ExternalInput")
    bm = nc.dram_tensor("bm", [n_blocks, _D], mybir.dt.float32, kind="ExternalOutput")

    # Chunk c = contiguous rows [c*P*CR, (c+1)*P*CR); partition p takes the
    # p-th CR-row run inside it.  Sequential block index = c*P + p.
    xc = x[:].rearrange("(c p w) d -> c p (w d)", c=n_chunks, p=_P)
    # Whole-sweep store view: partition p, then chunk-major, d inner.
    bo = bm[:].rearrange("(c p) d -> p c d", c=n_chunks, p=_P)

    total = repeats * n_chunks
    sw = n_chunks * _D  # bmt elements per sweep half

    with contextlib.ExitStack() as es:
        tiles = es.enter_context(nc.sbuf_tensor([_P, bufs * cw], mybir.dt.float32))
        bmt = es.enter_context(nc.sbuf_tensor([_P, 2 * sw], mybir.dt.float32))
        # One DMA-completion sem per buffer slot: at most one in-flight DMA
        # per sem, so `sem >= 16*k` exactly means "the k-th load into this
        # slot fully landed" (a single cumulative sem could pass its
        # threshold early if SDMA engines progress unevenly across chunks).
        dma_sems = [
            es.enter_context(nc.semaphore(f"dma_sem{i}")) for i in range(bufs)
        ]
        store_sem = es.enter_context(nc.semaphore("store_sem"))
        red_sem = es.enter_context(nc.semaphore("red_sem"))
        block = es.enter_context(nc.Block())

        # per-slot ordinal of each load (for the slot's completion sem)
        ordinal = [0] * bufs
        ords = {}
        for g in range(total):
            ordinal[g % bufs] += 1
            ords[g] = ordinal[g % bufs]

        @block.sync
        def _(sync):
            for g in range(total):
                if g >= bufs:
                    # the previous tenant of this slot has been folded
                    # (which also implies that load fully landed)
                    sync.wait_ge(red_sem, g - bufs + 1)
                slot = (g % bufs) * cw
                sync.dma_start(
                    out=tiles[:, slot : slot + cw], in_=xc[g % n_chunks]
                ).then_inc(dma_sems[g % bufs], 16)
            sync.wait_ge(store_sem, 16 * repeats)

        @block.scalar
        def _(scalar):
            for r in range(repeats):
                scalar.wait_ge(red_sem, n_chunks * (r + 1))
                half = (r % 2) * sw
                scalar.dma_start(
                    out=bo, in_=bmt[:, half : half + sw]
                ).then_inc(store_sem, 16)

        @block.vector
        def _(vector):
            for g in range(total):
                r, c = divmod(g, n_chunks)
                if r >= 2 and c == 0:
                    # bmt WAR: this parity half was last read by the store
                    # of sweep r-2
                    vector.wait_ge(store_sem, 16 * (r - 1))
                vector.wait_ge(dma_sems[g % bufs], 16 * ords[g])
                slot = (g % bufs) * cw
                half = (r % 2) * sw
                # in-place unit-stride fold tree: rows [0,m/2) vs [m/2,m)
                m = chunk_rows
                while m > 2:
                    h = (m // 2) * _D
                    nc.vector.tensor_max(
                        out=tiles[:, slot : slot + h],
                        in0=tiles[:, slot : slot + h],
                        in1=tiles[:, slot + h : slot + 2 * h],
                    )
                    m //= 2
                nc.vector.tensor_max(
                    out=bmt[:, half + c * _D : half + (c + 1) * _D],
                    in0=tiles[:, slot : slot + _D],
                    in1=tiles[:, slot + _D : slot + 2 * _D],
                ).then_inc(red_sem, 1)
    return nc


def _device_block_max(x):
    from concourse.bass_utils import run_bass_kernel_spmd

    n = x.shape[0]
    rows_per_core = n // _NCORES
    if rows_per_core not in _CACHE:
        _CACHE[rows_per_core] = _build_nc(rows_per_core)
    nc = _CACHE[rows_per_core]
    shards = [x[i * rows_per_core : (i + 1) * rows_per_core] for i in range(_NCORES)]
    res = run_bass_kernel_spmd(
        nc, [{"x": s} for s in shards], core_ids=list(range(_NCORES))
    )
    return np.concatenate([r["bm"] for r in res.results], axis=0)


def _combine(bm, x, batch, num_segments):
    n, d = x.shape
    counts = np.bincount(batch, minlength=num_segments)
    starts = np.empty(num_segments + 1, np.int64)
    starts[0] = 0
    np.cumsum(counts, out=starts[1:])

    rows_per_seg = n // num_segments if num_segments else 0
    if (
        num_segments
        and n % num_segments == 0
        and rows_per_seg % _K == 0
        and np.all(counts == rows_per_seg)
    ):
        return np.ascontiguousarray(
            bm.reshape(num_segments, rows_per_seg // _K, d).max(axis=1)
        )

    out = np.full((num_segments, d), -np.inf, dtype=np.float32)
    for s in range(num_segments):
        a, b = int(starts[s]), int(starts[s + 1])
        if a >= b:
            continue
        ca, cb = -(-a // _K), b // _K
        best = None
        if ca < cb:
            best = bm[ca:cb].max(axis=0)
        lo_end = min(b, ca * _K)
        if a < lo_end:
            e = x[a:lo_end].max(axis=0)
            best = e if best is None else np.maximum(best, e)
        hi_start = max(a, cb * _K)
        if hi_start < b:
            e = x[hi_start:b].max(axis=0)
            best = e if best is None else np.maximum(best, e)
        out[s] = best
    return out


def _numpy_segment_max(x, batch, num_segments):
    """Pure-host fallback for inputs the device path doesn't cover
    (unsorted batch, out-of-range ids, unexpected shapes)."""
    out = np.full((num_segments, x.shape[1]), -np.inf, dtype=np.float32)
    if batch.size == 0 or num_segments == 0:
        return out
    keep = (batch >= 0) & (batch < num_segments)
    xb, bb = x[keep], batch[keep]
    order = np.argsort(bb, kind="stable")
    xb, bb = xb[order], bb[order]
    counts = np.bincount(bb, minlength=num_segments)
    starts = np.concatenate([[0], np.cumsum(counts)[:-1]])
    nonempty = counts > 0
    idx = starts[nonempty]
    if idx.size:
        out[nonempty] = np.maximum.reduceat(xb, idx, axis=0)
    return out


def kernel(x, batch, num_segments):
    x = np.ascontiguousarray(np.asarray(x), dtype=np.float32)
    batch = np.asarray(batch)
    num_segments = int(np.asarray(num_segments))
    n, d = x.shape

    in_range = batch.size == 0 or (
        int(batch[0]) >= 0 and int(batch[-1]) < num_segments
    )
    sorted_ok = batch.size == 0 or bool(np.all(batch[1:] >= batch[:-1]))
    shape_ok = d == _D and n == batch.shape[0] and n % (_NCORES * _P * _CHUNK_ROWS) == 0

    if not (shape_ok and sorted_ok and in_range):
        return _numpy_segment_max(x, batch, num_segments)

    bm = _device_block_max(x)
    return _combine(bm, x, batch, num_segments)
